# revision 40
# baseline (speedup 1.0000x reference)
"""Causal dot-product attention (B=4, H=8, S=2048, DK=64) on 8 Trainium2 cores.

Sharding: B*H = 32 head-slices, 4 per core (pure data/head parallel, no
cross-device communication). Each core runs the same Bass/Tile program on its
own 4 heads; kernel() shards on the host, runs SPMD via
bass_utils.run_bass_kernel_spmd, and re-assembles the full output.

Per-head device algorithm (scores^T layout: k on partitions, q on free dim):
  1. DMA Q, K, V [2048, 64] fp32 into SBUF as 16 blocks of [128, 64].
     V is stored bf16 as V' [128, 16, 65] with a ones column appended -> the
     PV matmul produces the softmax denominators for free (row 64 of O'^T).
  2. PE-transpose Q and K blocks into bf16 Q^T, K^T [64, 2048] (d on
     partitions; the PSUM->SBUF copy performs the fp32->bf16 cast).
  3. Two passes per head (one per 1024-wide q-window; only one PSUM O'^T
     accumulator is live). Per k-tile i (128 keys), causally sliced:
       scores^T = (K^T tile)^T @ Q^T  -- all-bf16 matmuls in <=512-col
       pieces (PSUM bank limit). bf16 everywhere keeps the k-loop a dense
       bf16 MATMUL stream so the PE HAM stays un-throttled at 2.4 GHz.
  4. exp on ScalarE reading PSUM, scale=1/sqrt(dk) folded in, bf16 out.
     No max-subtraction needed: scores ~ N(0,1), exp is safe in fp32.
     The diagonal block's masked (q < k) entries are zeroed on VectorE by
     multiplying with a lower-triangular 0/1 constant.
  5. PV: O'^T [65, q] += V'^T @ P^T accumulated in PSUM over k-tiles,
     software-pipelined one k-tile behind QK so the PE never blocks on exp.
  6. Epilogue: copy O'^T to SBUF, PE-transpose back to [q, 65] blocks,
     reciprocal of column 64 (denominators), tensor_scalar normalize, DMA out.

Cross-head software pipeline: head h+1's loads/casts/transposes and head
h-1's epilogue are scattered one instruction-unit at a time through head h's
k-loop, keeping TensorE's instruction stream dense (~145-160 us/core measured,
vs 224 us for the naive schedule; exp on ScalarE ~83 us is the next floor).

Numerics: P, V, Q, K participate in matmuls as bf16 (fp32 accumulation).
Measured absmax error vs the fp32 reference is ~1.2e-2 on an output scale of
~3.5 (rel ~3.4e-3), dominated by bf16 rounding of P and V; bf16 Q/K adds
almost nothing (softmax weight errors largely cancel in the weighted sum).

Fallbacks in kernel(): a causal mask (or dk != 64) just re-parameterizes the
program; an all-zeros mask builds a non-causal variant; any other mask falls
back to a host fp64 reference implementation.
"""

import os
import sys

for _p in ("/opt/trn_rl_repo", "/opt/pypackages"):
    if _p not in sys.path:
        sys.path.insert(0, _p)

import numpy as np

B, H, S, DK = 4, 8, 2048, 64
NCORES = 8
HPC = (B * H) // NCORES  # heads per core
NB = S // 128  # 16 key tiles / q blocks
QW = 1024  # q-window width (2 PSUM banks)

_CACHE = {}
LAST_RESULT = None  # BassKernelResults of the most recent device run


def _split_bank_pieces(cs, ce, w0):
    """Split absolute col range [cs, ce) into matmul pieces that do not cross
    the 512-aligned PSUM bank boundaries of the window starting at w0."""
    pieces = []
    c = cs
    while c < ce:
        bank_end = w0 + ((c - w0) // 512 + 1) * 512
        pieces.append((c, min(ce, bank_end)))
        c = min(ce, bank_end)
    return pieces


def _build_program(causal=True, scale=0.125, sim_safe=False):
    # sim_safe: emit the diagonal PV columns as their own sub-piece with
    # stop=True so CoreSim's per-element accumulation-group tracking closes
    # them at the right k-tile. On hardware `stop` is a no-op (the math is
    # identical), so the deployed build merges them into the bank piece and
    # saves 16 matmuls per head.
    import concourse.bass as bass
    import concourse.mybir as mybir
    import concourse.tile as tile
    from concourse import bacc
    from concourse.masks import make_identity

    f32 = mybir.dt.float32
    bf16 = mybir.dt.bfloat16

    nc = bacc.Bacc("TRN2", target_bir_lowering=False)
    q = nc.dram_tensor("q", [HPC, S, DK], f32, kind="ExternalInput")
    k = nc.dram_tensor("k", [HPC, S, DK], f32, kind="ExternalInput")
    v = nc.dram_tensor("v", [HPC, S, DK], f32, kind="ExternalInput")
    o = nc.dram_tensor("o", [HPC, S, DK], f32, kind="ExternalOutput")

    with tile.TileContext(nc) as tc:
        from contextlib import ExitStack

        with ExitStack() as ctx:
            consts = ctx.enter_context(tc.tile_pool(name="consts", bufs=1))
            io = ctx.enter_context(tc.tile_pool(name="io", bufs=2))
            qtp = ctx.enter_context(tc.tile_pool(name="qtp", bufs=2))
            ptp = ctx.enter_context(tc.tile_pool(name="ptp", bufs=4))
            outp = ctx.enter_context(tc.tile_pool(name="outp", bufs=2))
            ps = ctx.enter_context(tc.tile_pool(name="ps", bufs=2, space="PSUM"))
            oap = ctx.enter_context(tc.tile_pool(name="oap", bufs=1, space="PSUM"))
            trp_pool = ctx.enter_context(
                tc.tile_pool(name="trp_pool", bufs=2, space="PSUM")
            )

            # constants
            ident = consts.tile([128, 128], f32)
            make_identity(nc, ident)
            # tri[p, c] = 1 where c >= p (q >= k allowed), else 0 -- zeros the
            # masked upper part of the diagonal P^T block on DVE
            tri = consts.tile([128, 128], bf16)
            nc.gpsimd.memset(tri, 1.0)
            nc.gpsimd.affine_select(
                out=tri,
                in_=tri,
                compare_op=mybir.AluOpType.is_ge,
                fill=0.0,
                base=0,
                pattern=[[1, 128]],
                channel_multiplier=-1,
            )

            tiles = {}  # per-head SBUF tiles

            def emit_loads(h):
                qin = io.tile([128, NB, DK], f32, tag="qin", name=f"qin{h}")
                kin = io.tile([128, NB, DK], f32, tag="kin", name=f"kin{h}")
                vpf = io.tile([128, NB, DK], f32, tag="vpf", name=f"vpf{h}")
                vp = io.tile([128, NB, DK + 1], bf16, tag="vp", name=f"vp{h}")
                qt2 = qtp.tile([DK, S], bf16, tag="qt", name=f"qt{h}")
                kt2 = qtp.tile([DK, S], bf16, tag="kt", name=f"kt{h}")
                for src_t, dst_t, eng in (
                    (q, qin, nc.sync),
                    (k, kin, nc.scalar),
                    (v, vpf, nc.sync),
                ):
                    rr = src_t[h].rearrange("(b p) d -> p b d", p=128)
                    eng.dma_start(dst_t[:, 0:8, :], rr[:, 0:8, :])
                    eng.dma_start(dst_t[:, 8:NB, :], rr[:, 8:NB, :])
                tiles[h] = dict(qin=qin, kin=kin, vpf=vpf, vp=vp, qt2=qt2, kt2=kt2)

            def prologue_units(h):
                """Single-op closures, scattered through the previous head's
                k-loop so the dense bf16 matmul stream keeps the PE HAM
                un-throttled."""
                t = tiles[h]

                def vp_unit():
                    nc.gpsimd.tensor_copy(t["vp"][:, :, 0:DK], t["vpf"])
                    nc.gpsimd.memset(t["vp"][:, :, DK], 1.0)

                state = {}

                def tr_unit(dst_name, src_name, grp, j):
                    def run():
                        key = (dst_name, grp)
                        if j == 0:
                            state[key] = trp_pool.tile(
                                [DK, 512], f32, tag="tr",
                                name=f"tr{h}{dst_name}{grp}",
                            )
                        ptr = state[key]
                        b = 4 * grp + j
                        nc.tensor.transpose(
                            ptr[:, 128 * j : 128 * (j + 1)],
                            t[src_name][:, b, :],
                            ident,
                        )
                        if j == 3:
                            dst = t[dst_name]
                            nc.vector.tensor_copy(
                                dst[:, 512 * grp : 512 * (grp + 1)], ptr
                            )

                    return run

                groups = {"vp": [vp_unit]}
                for gname, dst, srcf in (("q", "qt2", "qin"), ("k", "kt2", "kin")):
                    for grp in range(4):
                        groups[f"{gname}{grp}"] = [
                            tr_unit(dst, srcf, grp, j) for j in range(4)
                        ]
                return groups

            def epilogue_units(h, groups):
                """Transpose+normalize groups (2 q-blocks each... 4 blocks)"""
                t = tiles[h]
                osb, ot, rt = t["osb"], t["ot"], t["rt"]
                units = []
                for g in groups:

                    def ep_unit(g=g):
                        trp = trp_pool.tile(
                            [128, 4, DK + 1], f32, tag="tr", name=f"ep{h}{g}"
                        )
                        for j in range(4):
                            b = 4 * g + j
                            nc.tensor.transpose(
                                trp[:, j, :],
                                osb[:, 128 * b : 128 * (b + 1)],
                                ident[0 : DK + 1, 0 : DK + 1],
                            )
                        nc.vector.reciprocal(rt[:, 4 * g : 4 * g + 4], trp[:, :, DK])
                        for j in range(4):
                            b = 4 * g + j
                            nc.vector.tensor_scalar_mul(
                                ot[:, b, :], trp[:, j, 0:DK], rt[:, b : b + 1]
                            )

                    units.append(ep_unit)
                return units

            emit_loads(0)
            g0 = prologue_units(0)
            # upfront: vp + q quarters 0-1 + k quarter 0; the rest
            # interleaves into head 0's own k-loop ordered by first use:
            # k q1 (iter 4), q q2/q3 (pass-1 start, iter 8), k q2/q3
            # (iters 16/20); consumed 2 per iteration.
            for u in g0["vp"] + g0["q0"] + g0["q1"] + g0["k0"]:
                u()
            leftover0 = (
                g0["k1"] + g0["q2"] + g0["q3"] + g0["k2"] + g0["k3"]
            )
            pending_ep = []

            for h in range(HPC):
                t = tiles[h]
                qt2, kt2, vp = t["qt2"], t["kt2"], t["vp"]
                t["osb"] = outp.tile([DK + 1, S], f32, tag="osb", name=f"osb{h}")
                t["ot"] = outp.tile([128, NB, DK], f32, tag="ot", name=f"ot{h}")
                t["rt"] = outp.tile([128, NB], f32, tag="rt", name=f"rt{h}")
                osb = t["osb"]
                pending_pro = []
                it_count = 0

                for wi in range(2):
                    w0 = QW * wi
                    ce = w0 + QW
                    ilist = [
                        i for i in range(NB) if not (causal and w0 + QW <= 128 * i)
                    ]
                    last_i = ilist[-1]
                    oacc = oap.tile([DK + 1, QW], f32, tag="oacc", name=f"oacc{h}{wi}")
                    pending_pv = []

                    def emit_pv(pend, oacc=oacc, w0=w0, first_i=None):
                        pi_, pt_, pieces_, fi = pend
                        for a, bnd, stop_f in pieces_:
                            nc.tensor.matmul(
                                oacc[:, a - w0 : bnd - w0],
                                vp[:, pi_, :],
                                pt_[:, a - w0 : bnd - w0],
                                start=fi,
                                stop=stop_f,
                                skip_group_check=True,
                            )

                    for i in ilist:
                        # interleave cross-head work into the dense stream
                        if it_count == 0:
                            if h == 0:
                                pending_pro = list(leftover0)
                            if h + 1 < HPC:
                                emit_loads(h + 1)
                                gs = prologue_units(h + 1)
                                pending_pro = pending_pro + [
                                    u
                                    for key in (
                                        "vp", "q0", "q1", "q2", "q3",
                                        "k0", "k1", "k2", "k3",
                                    )
                                    for u in gs[key]
                                ]
                        if pending_pro:
                            pending_pro.pop(0)()
                            if (h == 0 or it_count >= 10) and pending_pro:
                                pending_pro.pop(0)()
                        if it_count % 3 == 2 and pending_ep:
                            pending_ep.pop(0)()
                        it_count += 1

                        kstart = 128 * i
                        cs = max(w0, kstart) if causal else w0
                        sc = ps.tile([128, QW], f32, tag="ps", name=f"sc{h}{wi}{i}")
                        pieces = _split_bank_pieces(cs, ce, w0)
                        for a, bnd in pieces:
                            nc.tensor.matmul(
                                sc[:, a - w0 : bnd - w0],
                                kt2[:, kstart : kstart + 128],
                                qt2[:, a:bnd],
                                start=True,
                                stop=True,
                                skip_group_check=True,
                            )
                        pt = ptp.tile([128, QW], bf16, tag="pt", name=f"pt{h}{wi}{i}")
                        nc.scalar.activation(
                            pt[:, cs - w0 : QW],
                            sc[:, cs - w0 : QW],
                            mybir.ActivationFunctionType.Exp,
                            scale=scale,
                        )
                        if causal and cs == kstart:
                            # zero masked (q < k) entries of the diagonal block
                            nc.vector.tensor_mul(
                                pt[:, cs - w0 : cs - w0 + 128],
                                pt[:, cs - w0 : cs - w0 + 128],
                                tri,
                            )
                        if pending_pv:
                            emit_pv(pending_pv.pop(0))
                        if not sim_safe and i == 8 * wi + 5:
                            # cols [w0, w0+512) got their last PV at k-tile
                            # 8*wi+3 (popped at iteration 8*wi+4): copy that
                            # half out now so epilogue groups start earlier
                            nc.vector.tensor_copy(
                                osb[:, w0 : w0 + 512], oacc[:, 0:512]
                            )
                        if causal and cs == kstart and sim_safe:
                            pv_pieces = [(cs, cs + 128, True)]
                            pv_pieces += [
                                (a, bnd, i == last_i)
                                for a, bnd in _split_bank_pieces(cs + 128, ce, w0)
                            ]
                        else:
                            pv_pieces = [(a, bnd, i == last_i) for a, bnd in pieces]
                        pending_pv.append((i, pt, pv_pieces, i == ilist[0]))
                    for pend in pending_pv:
                        emit_pv(pend)
                    if sim_safe:
                        nc.vector.tensor_copy(osb[:, w0:ce], oacc)
                    else:
                        nc.vector.tensor_copy(
                            osb[:, w0 + 512 : ce], oacc[:, 512:QW]
                        )
                    # epilogue groups for this pass's q-blocks become eligible
                    pending_ep.extend(epilogue_units(h, [2 * wi, 2 * wi + 1]))
                    if wi == 1:
                        hh = h

                        def out_dma(hh=hh):
                            nc.scalar.dma_start(
                                o[hh].rearrange("(b p) d -> p b d", p=128),
                                tiles[hh]["ot"],
                            )

                        pending_ep.append(out_dma)

                for u in pending_pro:
                    u()
            for u in pending_ep:
                u()

    nc.compile()
    return nc


def _get_program(causal, scale):
    key = (causal, float(scale))
    if key not in _CACHE:
        _CACHE[key] = _build_program(causal=causal, scale=scale)
    return _CACHE[key]


def _mask_kind(mask):
    """'causal' | 'none' | 'other'"""
    if mask is None:
        return "none"
    m = np.asarray(mask)
    if m.size == 0 or not np.any(m):
        return "none"
    m2 = m.reshape(m.shape[-2], m.shape[-1])
    tri = np.triu(np.ones((S, S), dtype=m2.dtype), k=1)
    if m2.shape == (S, S) and np.array_equal(m2, tri):
        return "causal"
    return "other"


def _host_reference(queries, keys, values, dk, mask):
    """Correctness fallback for mask shapes the device program doesn't cover."""
    q = queries.astype(np.float64)
    kk = keys.astype(np.float64)
    vv = values.astype(np.float64)
    score = np.einsum("bhqd,bhkd->bhqk", q, kk) / np.sqrt(np.float64(dk))
    if mask is not None:
        score = score + np.asarray(mask, dtype=np.float64) * -1e9
    score -= score.max(axis=-1, keepdims=True)
    e = np.exp(score)
    attn = e / e.sum(axis=-1, keepdims=True)
    return np.einsum("bhqk,bhkd->bhqd", attn, vv).astype(np.float32)


def kernel(queries, keys, values, dk, mask=None, **_kw):
    global LAST_RESULT
    dk_val = int(np.asarray(dk))
    kind = _mask_kind(mask)
    if kind == "other":
        return _host_reference(queries, keys, values, dk_val, mask)

    from concourse.bass_utils import run_bass_kernel_spmd

    scale = 1.0 / float(np.sqrt(np.float64(dk_val)))
    nc = _get_program(causal=(kind == "causal"), scale=scale)

    qf = np.ascontiguousarray(
        np.asarray(queries, dtype=np.float32).reshape(B * H, S, DK)
    )
    kf = np.ascontiguousarray(np.asarray(keys, dtype=np.float32).reshape(B * H, S, DK))
    vf = np.ascontiguousarray(
        np.asarray(values, dtype=np.float32).reshape(B * H, S, DK)
    )

    in_maps = [
        {
            "q": qf[HPC * c : HPC * (c + 1)],
            "k": kf[HPC * c : HPC * (c + 1)],
            "v": vf[HPC * c : HPC * (c + 1)],
        }
        for c in range(NCORES)
    ]
    res = run_bass_kernel_spmd(nc, in_maps, core_ids=list(range(NCORES)))
    LAST_RESULT = res
    out = np.stack([res.results[c]["o"] for c in range(NCORES)], axis=0)
    return out.reshape(B, H, S, DK).astype(np.float32)


if __name__ == "__main__":
    # smoke: build the program only
    nc = _build_program()
    print("program built ok")


# revision 41
# speedup vs baseline: 1.0897x; 1.0897x over previous
"""Causal dot-product attention (B=4, H=8, S=2048, DK=64) on 8 Trainium2 cores.

Sharding: B*H = 32 head-slices, 4 per core (pure data/head parallel, no
cross-device communication). Each core runs the same Bass/Tile program on its
own 4 heads; kernel() shards on the host, runs SPMD via
bass_utils.run_bass_kernel_spmd, and re-assembles the full output.

Per-head device algorithm (scores^T layout: k on partitions, q on free dim):
  1. DMA Q, K, V [2048, 64] fp32 into SBUF as 16 blocks of [128, 64].
     V is stored bf16 as V' [128, 16, 65] with a ones column appended -> the
     PV matmul produces the softmax denominators for free (row 64 of O'^T).
  2. PE-transpose Q and K blocks into bf16 Q^T, K^T [64, 2048] (d on
     partitions; the PSUM->SBUF copy performs the fp32->bf16 cast).
  3. Two passes per head (one per 1024-wide q-window; only one PSUM O'^T
     accumulator is live). Per k-tile i (128 keys), causally sliced:
       scores^T = (K^T tile)^T @ Q^T  -- all-bf16 matmuls in <=512-col
       pieces (PSUM bank limit). bf16 everywhere keeps the k-loop a dense
       bf16 MATMUL stream so the PE HAM stays un-throttled at 2.4 GHz.
  4. exp on ScalarE reading PSUM, scale=1/sqrt(dk) folded in, bf16 out.
     No max-subtraction needed: scores ~ N(0,1), exp is safe in fp32.
     The diagonal block's masked (q < k) entries are zeroed on VectorE by
     multiplying with a lower-triangular 0/1 constant.
  5. PV: O'^T [65, q] += V'^T @ P^T accumulated in PSUM over k-tiles,
     software-pipelined one k-tile behind QK so the PE never blocks on exp.
  6. Epilogue: copy O'^T to SBUF, PE-transpose back to [q, 65] blocks,
     reciprocal of column 64 (denominators), tensor_scalar normalize, DMA out.

Cross-head software pipeline: head h+1's loads/casts/transposes and head
h-1's epilogue are scattered one instruction-unit at a time through head h's
k-loop, keeping TensorE's instruction stream dense (~145-160 us/core measured,
vs 224 us for the naive schedule; exp on ScalarE ~83 us is the next floor).

Numerics: P, V, Q, K participate in matmuls as bf16 (fp32 accumulation).
Measured absmax error vs the fp32 reference is ~1.2e-2 on an output scale of
~3.5 (rel ~3.4e-3), dominated by bf16 rounding of P and V; bf16 Q/K adds
almost nothing (softmax weight errors largely cancel in the weighted sum).

Fallbacks in kernel(): a causal mask (or dk != 64) just re-parameterizes the
program; an all-zeros mask builds a non-causal variant; any other mask falls
back to a host fp64 reference implementation.
"""

import os
import sys

for _p in ("/opt/trn_rl_repo", "/opt/pypackages"):
    if _p not in sys.path:
        sys.path.insert(0, _p)

import numpy as np

B, H, S, DK = 4, 8, 2048, 64
NCORES = 8
HPC = (B * H) // NCORES  # heads per core
NB = S // 128  # 16 key tiles / q blocks
QW = 1024  # q-window width (2 PSUM banks)

_CACHE = {}
LAST_RESULT = None  # BassKernelResults of the most recent device run


def _split_bank_pieces(cs, ce, w0):
    """Split absolute col range [cs, ce) into matmul pieces that do not cross
    the 512-aligned PSUM bank boundaries of the window starting at w0."""
    pieces = []
    c = cs
    while c < ce:
        bank_end = w0 + ((c - w0) // 512 + 1) * 512
        pieces.append((c, min(ce, bank_end)))
        c = min(ce, bank_end)
    return pieces


def _build_program(causal=True, scale=0.125, sim_safe=False):
    # sim_safe: emit the diagonal PV columns as their own sub-piece with
    # stop=True so CoreSim's per-element accumulation-group tracking closes
    # them at the right k-tile. On hardware `stop` is a no-op (the math is
    # identical), so the deployed build merges them into the bank piece and
    # saves 16 matmuls per head.
    import concourse.bass as bass
    import concourse.mybir as mybir
    import concourse.tile as tile
    from concourse import bacc
    from concourse.masks import make_identity

    f32 = mybir.dt.float32
    bf16 = mybir.dt.bfloat16

    nc = bacc.Bacc("TRN2", target_bir_lowering=False)
    q = nc.dram_tensor("q", [HPC, S, DK], f32, kind="ExternalInput")
    k = nc.dram_tensor("k", [HPC, S, DK], f32, kind="ExternalInput")
    v = nc.dram_tensor("v", [HPC, S, DK], f32, kind="ExternalInput")
    o = nc.dram_tensor("o", [HPC, S, DK], f32, kind="ExternalOutput")

    with tile.TileContext(nc) as tc:
        from contextlib import ExitStack

        with ExitStack() as ctx:
            consts = ctx.enter_context(tc.tile_pool(name="consts", bufs=1))
            io = ctx.enter_context(tc.tile_pool(name="io", bufs=2))
            qtp = ctx.enter_context(tc.tile_pool(name="qtp", bufs=2))
            ptp = ctx.enter_context(tc.tile_pool(name="ptp", bufs=4))
            outp = ctx.enter_context(tc.tile_pool(name="outp", bufs=2))
            ps = ctx.enter_context(tc.tile_pool(name="ps", bufs=2, space="PSUM"))
            oap = ctx.enter_context(tc.tile_pool(name="oap", bufs=1, space="PSUM"))
            trp_pool = ctx.enter_context(
                tc.tile_pool(name="trp_pool", bufs=2, space="PSUM")
            )

            # constants
            ident = consts.tile([128, 128], f32)
            make_identity(nc, ident)
            # tri[p, c] = 1 where c >= p (q >= k allowed), else 0 -- zeros the
            # masked upper part of the diagonal P^T block on DVE
            tri = consts.tile([128, 128], bf16)
            nc.gpsimd.memset(tri, 1.0)
            nc.gpsimd.affine_select(
                out=tri,
                in_=tri,
                compare_op=mybir.AluOpType.is_ge,
                fill=0.0,
                base=0,
                pattern=[[1, 128]],
                channel_multiplier=-1,
            )

            tiles = {}  # per-head SBUF tiles

            def emit_loads(h):
                qin = io.tile([128, NB, DK], f32, tag="qin", name=f"qin{h}")
                kin = io.tile([128, NB, DK], f32, tag="kin", name=f"kin{h}")
                vpf = io.tile([128, NB, DK], f32, tag="vpf", name=f"vpf{h}")
                vp = io.tile([128, NB, DK + 1], bf16, tag="vp", name=f"vp{h}")
                qt2 = qtp.tile([DK, S], bf16, tag="qt", name=f"qt{h}")
                kt2 = qtp.tile([DK, S], bf16, tag="kt", name=f"kt{h}")
                for src_t, dst_t in ((q, qin), (k, kin), (v, vpf)):
                    rr = src_t[h].rearrange("(b p) d -> p b d", p=128)
                    nc.sync.dma_start(dst_t[:, 0:8, :], rr[:, 0:8, :])
                    nc.sync.dma_start(dst_t[:, 8:NB, :], rr[:, 8:NB, :])
                tiles[h] = dict(qin=qin, kin=kin, vpf=vpf, vp=vp, qt2=qt2, kt2=kt2)

            def prologue_units(h):
                """Single-op closures, scattered through the previous head's
                k-loop so the dense bf16 matmul stream keeps the PE HAM
                un-throttled."""
                t = tiles[h]

                def vp_unit():
                    nc.gpsimd.tensor_copy(t["vp"][:, :, 0:DK], t["vpf"])
                    nc.gpsimd.memset(t["vp"][:, :, DK], 1.0)

                state = {}

                def tr_unit(dst_name, src_name, grp, j):
                    def run():
                        key = (dst_name, grp)
                        if j == 0:
                            state[key] = trp_pool.tile(
                                [DK, 512], f32, tag="tr",
                                name=f"tr{h}{dst_name}{grp}",
                            )
                        ptr = state[key]
                        b = 4 * grp + j
                        nc.tensor.transpose(
                            ptr[:, 128 * j : 128 * (j + 1)],
                            t[src_name][:, b, :],
                            ident,
                        )
                        if j == 3:
                            dst = t[dst_name]
                            nc.vector.tensor_copy(
                                dst[:, 512 * grp : 512 * (grp + 1)], ptr
                            )

                    return run

                groups = {"vp": [vp_unit]}
                for gname, dst, srcf in (("q", "qt2", "qin"), ("k", "kt2", "kin")):
                    for grp in range(4):
                        groups[f"{gname}{grp}"] = [
                            tr_unit(dst, srcf, grp, j) for j in range(4)
                        ]
                return groups

            def epilogue_units(h, groups):
                """Transpose+normalize groups (2 q-blocks each... 4 blocks)"""
                t = tiles[h]
                osb, ot, rt = t["osb"], t["ot"], t["rt"]
                units = []
                for g in groups:

                    def ep_unit(g=g):
                        trp = trp_pool.tile(
                            [128, 4, DK + 1], f32, tag="tr", name=f"ep{h}{g}"
                        )
                        for j in range(4):
                            b = 4 * g + j
                            nc.tensor.transpose(
                                trp[:, j, :],
                                osb[:, 128 * b : 128 * (b + 1)],
                                ident[0 : DK + 1, 0 : DK + 1],
                            )
                        nc.vector.reciprocal(rt[:, 4 * g : 4 * g + 4], trp[:, :, DK])
                        for j in range(4):
                            b = 4 * g + j
                            nc.vector.tensor_scalar_mul(
                                ot[:, b, :], trp[:, j, 0:DK], rt[:, b : b + 1]
                            )

                    units.append(ep_unit)
                return units

            emit_loads(0)
            g0 = prologue_units(0)
            # upfront: vp + q quarters 0-1 + k quarter 0; the rest
            # interleaves into head 0's own k-loop ordered by first use:
            # k q1 (iter 4), q q2/q3 (pass-1 start, iter 8), k q2/q3
            # (iters 16/20); consumed 2 per iteration.
            for u in g0["vp"] + g0["q0"] + g0["q1"] + g0["k0"]:
                u()
            leftover0 = (
                g0["k1"] + g0["q2"] + g0["q3"] + g0["k2"] + g0["k3"]
            )
            pending_ep = []

            for h in range(HPC):
                t = tiles[h]
                qt2, kt2, vp = t["qt2"], t["kt2"], t["vp"]
                t["osb"] = outp.tile([DK + 1, S], f32, tag="osb", name=f"osb{h}")
                t["ot"] = outp.tile([128, NB, DK], f32, tag="ot", name=f"ot{h}")
                t["rt"] = outp.tile([128, NB], f32, tag="rt", name=f"rt{h}")
                osb = t["osb"]
                pending_pro = []
                it_count = 0

                for wi in range(2):
                    w0 = QW * wi
                    ce = w0 + QW
                    ilist = [
                        i for i in range(NB) if not (causal and w0 + QW <= 128 * i)
                    ]
                    last_i = ilist[-1]
                    oacc = oap.tile([DK + 1, QW], f32, tag="oacc", name=f"oacc{h}{wi}")
                    pending_pv = []

                    def emit_pv(pend, oacc=oacc, w0=w0, first_i=None):
                        pi_, pt_, pieces_, fi = pend
                        for a, bnd, stop_f in pieces_:
                            nc.tensor.matmul(
                                oacc[:, a - w0 : bnd - w0],
                                vp[:, pi_, :],
                                pt_[:, a - w0 : bnd - w0],
                                start=fi,
                                stop=stop_f,
                                skip_group_check=True,
                            )

                    for i in ilist:
                        # interleave cross-head work into the dense stream
                        if it_count == 0:
                            if h == 0:
                                pending_pro = list(leftover0)
                            if h + 1 < HPC:
                                emit_loads(h + 1)
                                gs = prologue_units(h + 1)
                                pending_pro = pending_pro + [
                                    u
                                    for key in (
                                        "vp", "q0", "q1", "q2", "q3",
                                        "k0", "k1", "k2", "k3",
                                    )
                                    for u in gs[key]
                                ]
                        if pending_pro:
                            pending_pro.pop(0)()
                            if (h == 0 or it_count >= 10) and pending_pro:
                                pending_pro.pop(0)()
                        if it_count % 3 == 2 and pending_ep:
                            pending_ep.pop(0)()
                        it_count += 1

                        kstart = 128 * i
                        cs = max(w0, kstart) if causal else w0
                        sc = ps.tile([128, QW], f32, tag="ps", name=f"sc{h}{wi}{i}")
                        pieces = _split_bank_pieces(cs, ce, w0)
                        for a, bnd in pieces:
                            nc.tensor.matmul(
                                sc[:, a - w0 : bnd - w0],
                                kt2[:, kstart : kstart + 128],
                                qt2[:, a:bnd],
                                start=True,
                                stop=True,
                                skip_group_check=True,
                            )
                        pt = ptp.tile([128, QW], bf16, tag="pt", name=f"pt{h}{wi}{i}")
                        nc.scalar.activation(
                            pt[:, cs - w0 : QW],
                            sc[:, cs - w0 : QW],
                            mybir.ActivationFunctionType.Exp,
                            scale=scale,
                        )
                        if causal and cs == kstart:
                            # zero masked (q < k) entries of the diagonal block
                            nc.vector.tensor_mul(
                                pt[:, cs - w0 : cs - w0 + 128],
                                pt[:, cs - w0 : cs - w0 + 128],
                                tri,
                            )
                        if pending_pv:
                            emit_pv(pending_pv.pop(0))
                        if not sim_safe and i == 8 * wi + 5:
                            # cols [w0, w0+512) got their last PV at k-tile
                            # 8*wi+3 (popped at iteration 8*wi+4): copy that
                            # half out now so epilogue groups start earlier
                            nc.vector.tensor_copy(
                                osb[:, w0 : w0 + 512], oacc[:, 0:512]
                            )
                        if causal and cs == kstart and sim_safe:
                            pv_pieces = [(cs, cs + 128, True)]
                            pv_pieces += [
                                (a, bnd, i == last_i)
                                for a, bnd in _split_bank_pieces(cs + 128, ce, w0)
                            ]
                        else:
                            pv_pieces = [(a, bnd, i == last_i) for a, bnd in pieces]
                        pending_pv.append((i, pt, pv_pieces, i == ilist[0]))
                    for pend in pending_pv:
                        emit_pv(pend)
                    if sim_safe:
                        nc.vector.tensor_copy(osb[:, w0:ce], oacc)
                    else:
                        nc.vector.tensor_copy(
                            osb[:, w0 + 512 : ce], oacc[:, 512:QW]
                        )
                    # epilogue groups for this pass's q-blocks become eligible
                    pending_ep.extend(epilogue_units(h, [2 * wi, 2 * wi + 1]))
                    if wi == 1:
                        hh = h

                        def out_dma(hh=hh):
                            nc.sync.dma_start(
                                o[hh].rearrange("(b p) d -> p b d", p=128),
                                tiles[hh]["ot"],
                            )

                        pending_ep.append(out_dma)

                for u in pending_pro:
                    u()
            for u in pending_ep:
                u()

    nc.compile()
    return nc


def _get_program(causal, scale):
    key = (causal, float(scale))
    if key not in _CACHE:
        _CACHE[key] = _build_program(causal=causal, scale=scale)
    return _CACHE[key]


def _mask_kind(mask):
    """'causal' | 'none' | 'other'"""
    if mask is None:
        return "none"
    m = np.asarray(mask)
    if m.size == 0 or not np.any(m):
        return "none"
    m2 = m.reshape(m.shape[-2], m.shape[-1])
    tri = np.triu(np.ones((S, S), dtype=m2.dtype), k=1)
    if m2.shape == (S, S) and np.array_equal(m2, tri):
        return "causal"
    return "other"


def _host_reference(queries, keys, values, dk, mask):
    """Correctness fallback for mask shapes the device program doesn't cover."""
    q = queries.astype(np.float64)
    kk = keys.astype(np.float64)
    vv = values.astype(np.float64)
    score = np.einsum("bhqd,bhkd->bhqk", q, kk) / np.sqrt(np.float64(dk))
    if mask is not None:
        score = score + np.asarray(mask, dtype=np.float64) * -1e9
    score -= score.max(axis=-1, keepdims=True)
    e = np.exp(score)
    attn = e / e.sum(axis=-1, keepdims=True)
    return np.einsum("bhqk,bhkd->bhqd", attn, vv).astype(np.float32)


def kernel(queries, keys, values, dk, mask=None, **_kw):
    global LAST_RESULT
    dk_val = int(np.asarray(dk))
    kind = _mask_kind(mask)
    if kind == "other":
        return _host_reference(queries, keys, values, dk_val, mask)

    from concourse.bass_utils import run_bass_kernel_spmd

    scale = 1.0 / float(np.sqrt(np.float64(dk_val)))
    nc = _get_program(causal=(kind == "causal"), scale=scale)

    qf = np.ascontiguousarray(
        np.asarray(queries, dtype=np.float32).reshape(B * H, S, DK)
    )
    kf = np.ascontiguousarray(np.asarray(keys, dtype=np.float32).reshape(B * H, S, DK))
    vf = np.ascontiguousarray(
        np.asarray(values, dtype=np.float32).reshape(B * H, S, DK)
    )

    in_maps = [
        {
            "q": qf[HPC * c : HPC * (c + 1)],
            "k": kf[HPC * c : HPC * (c + 1)],
            "v": vf[HPC * c : HPC * (c + 1)],
        }
        for c in range(NCORES)
    ]
    res = run_bass_kernel_spmd(nc, in_maps, core_ids=list(range(NCORES)))
    LAST_RESULT = res
    out = np.stack([res.results[c]["o"] for c in range(NCORES)], axis=0)
    return out.reshape(B, H, S, DK).astype(np.float32)


if __name__ == "__main__":
    # smoke: build the program only
    nc = _build_program()
    print("program built ok")


# revision 42
# speedup vs baseline: 1.1982x; 1.0996x over previous
"""Causal dot-product attention (B=4, H=8, S=2048, DK=64) on 8 Trainium2 cores.

Sharding: B*H = 32 head-slices, 4 per core (pure data/head parallel, no
cross-device communication). Each core runs the same Bass/Tile program on its
own 4 heads; kernel() shards on the host, runs SPMD via
bass_utils.run_bass_kernel_spmd, and re-assembles the full output.

Per-head device algorithm (scores^T layout: k on partitions, q on free dim):
  1. DMA Q, K, V [2048, 64] fp32 into SBUF as 16 blocks of [128, 64].
     V is stored bf16 as V' [128, 16, 65] with a ones column appended -> the
     PV matmul produces the softmax denominators for free (row 64 of O'^T).
  2. PE-transpose Q and K blocks into bf16 Q^T, K^T [64, 2048] (d on
     partitions; the PSUM->SBUF copy performs the fp32->bf16 cast).
  3. Two passes per head (one per 1024-wide q-window; only one PSUM O'^T
     accumulator is live). Per k-tile i (128 keys), causally sliced:
       scores^T = (K^T tile)^T @ Q^T  -- all-bf16 matmuls in <=512-col
       pieces (PSUM bank limit). bf16 everywhere keeps the k-loop a dense
       bf16 MATMUL stream so the PE HAM stays un-throttled at 2.4 GHz.
  4. exp on ScalarE reading PSUM, scale=1/sqrt(dk) folded in, bf16 out.
     No max-subtraction needed: scores ~ N(0,1), exp is safe in fp32.
     The diagonal block's masked (q < k) entries are zeroed on VectorE by
     multiplying with a lower-triangular 0/1 constant.
  5. PV: O'^T [65, q] += V'^T @ P^T accumulated in PSUM over k-tiles,
     software-pipelined one k-tile behind QK so the PE never blocks on exp.
  6. Epilogue: copy O'^T to SBUF, PE-transpose back to [q, 65] blocks,
     reciprocal of column 64 (denominators), tensor_scalar normalize, DMA out.

Cross-head software pipeline: head h+1's loads/casts/transposes and head
h-1's epilogue are scattered one instruction-unit at a time through head h's
k-loop, keeping TensorE's instruction stream dense (~145-160 us/core measured,
vs 224 us for the naive schedule; exp on ScalarE ~83 us is the next floor).

Numerics: P, V, Q, K participate in matmuls as bf16 (fp32 accumulation).
Measured absmax error vs the fp32 reference is ~1.2e-2 on an output scale of
~3.5 (rel ~3.4e-3), dominated by bf16 rounding of P and V; bf16 Q/K adds
almost nothing (softmax weight errors largely cancel in the weighted sum).

Fallbacks in kernel(): a causal mask (or dk != 64) just re-parameterizes the
program; an all-zeros mask builds a non-causal variant; any other mask falls
back to a host fp64 reference implementation.
"""

import os
import sys

for _p in ("/opt/trn_rl_repo", "/opt/pypackages"):
    if _p not in sys.path:
        sys.path.insert(0, _p)

import numpy as np

B, H, S, DK = 4, 8, 2048, 64
NCORES = 8
HPC = (B * H) // NCORES  # heads per core
NB = S // 128  # 16 key tiles / q blocks
QW = 1024  # q-window width (2 PSUM banks)

_CACHE = {}
LAST_RESULT = None  # BassKernelResults of the most recent device run


def _split_bank_pieces(cs, ce, w0):
    """Split absolute col range [cs, ce) into matmul pieces that do not cross
    the 512-aligned PSUM bank boundaries of the window starting at w0."""
    pieces = []
    c = cs
    while c < ce:
        bank_end = w0 + ((c - w0) // 512 + 1) * 512
        pieces.append((c, min(ce, bank_end)))
        c = min(ce, bank_end)
    return pieces


def _build_program(causal=True, scale=0.125, sim_safe=False):
    # sim_safe: emit the diagonal PV columns as their own sub-piece with
    # stop=True so CoreSim's per-element accumulation-group tracking closes
    # them at the right k-tile. On hardware `stop` is a no-op (the math is
    # identical), so the deployed build merges them into the bank piece and
    # saves 16 matmuls per head.
    import concourse.bass as bass
    import concourse.mybir as mybir
    import concourse.tile as tile
    from concourse import bacc
    from concourse.masks import make_identity

    f32 = mybir.dt.float32
    bf16 = mybir.dt.bfloat16

    nc = bacc.Bacc("TRN2", target_bir_lowering=False)
    q = nc.dram_tensor("q", [HPC, S, DK], f32, kind="ExternalInput")
    k = nc.dram_tensor("k", [HPC, S, DK], f32, kind="ExternalInput")
    v = nc.dram_tensor("v", [HPC, S, DK], f32, kind="ExternalInput")
    o = nc.dram_tensor("o", [HPC, S, DK], f32, kind="ExternalOutput")

    with tile.TileContext(nc) as tc:
        from contextlib import ExitStack

        with ExitStack() as ctx:
            consts = ctx.enter_context(tc.tile_pool(name="consts", bufs=1))
            io = ctx.enter_context(tc.tile_pool(name="io", bufs=2))
            qtp = ctx.enter_context(tc.tile_pool(name="qtp", bufs=2))
            ptp = ctx.enter_context(tc.tile_pool(name="ptp", bufs=4))
            outp = ctx.enter_context(tc.tile_pool(name="outp", bufs=2))
            ps = ctx.enter_context(tc.tile_pool(name="ps", bufs=2, space="PSUM"))
            oap = ctx.enter_context(tc.tile_pool(name="oap", bufs=1, space="PSUM"))
            trp_pool = ctx.enter_context(
                tc.tile_pool(name="trp_pool", bufs=2, space="PSUM")
            )

            # constants
            ident = consts.tile([128, 128], f32)
            make_identity(nc, ident)
            # causal mask applied on PE: scores += identb^T @ maskc over the
            # diagonal bank piece; maskc[p, c] = -1e9 where c < p (q < k),
            # zeros for c in [128, 512) so one matmul spans the whole piece.
            identb = consts.tile([128, 128], bf16)
            make_identity(nc, identb)
            maskc = consts.tile([128, 512], bf16)
            nc.gpsimd.memset(maskc, -1e9)
            nc.gpsimd.affine_select(
                out=maskc,
                in_=maskc,
                compare_op=mybir.AluOpType.is_gt,
                fill=0.0,
                base=0,
                pattern=[[-1, 512]],
                channel_multiplier=1,
            )

            tiles = {}  # per-head SBUF tiles

            def emit_loads(h):
                qin = io.tile([128, NB, DK], f32, tag="qin", name=f"qin{h}")
                kin = io.tile([128, NB, DK], f32, tag="kin", name=f"kin{h}")
                vpf = io.tile([128, NB, DK], f32, tag="vpf", name=f"vpf{h}")
                vp = io.tile([128, NB, DK + 1], bf16, tag="vp", name=f"vp{h}")
                qt2 = qtp.tile([DK, S], bf16, tag="qt", name=f"qt{h}")
                kt2 = qtp.tile([DK, S], bf16, tag="kt", name=f"kt{h}")
                for src_t, dst_t in ((q, qin), (k, kin), (v, vpf)):
                    rr = src_t[h].rearrange("(b p) d -> p b d", p=128)
                    nc.sync.dma_start(dst_t[:, 0:8, :], rr[:, 0:8, :])
                    nc.sync.dma_start(dst_t[:, 8:NB, :], rr[:, 8:NB, :])
                tiles[h] = dict(qin=qin, kin=kin, vpf=vpf, vp=vp, qt2=qt2, kt2=kt2)

            def prologue_units(h):
                """Single-op closures, scattered through the previous head's
                k-loop so the dense bf16 matmul stream keeps the PE HAM
                un-throttled."""
                t = tiles[h]

                def vp_unit():
                    nc.gpsimd.tensor_copy(t["vp"][:, :, 0:DK], t["vpf"])
                    nc.gpsimd.memset(t["vp"][:, :, DK], 1.0)

                state = {}

                def tr_unit(dst_name, src_name, grp, j):
                    def run():
                        key = (dst_name, grp)
                        if j == 0:
                            state[key] = trp_pool.tile(
                                [DK, 512], f32, tag="tr",
                                name=f"tr{h}{dst_name}{grp}",
                            )
                        ptr = state[key]
                        b = 4 * grp + j
                        nc.tensor.transpose(
                            ptr[:, 128 * j : 128 * (j + 1)],
                            t[src_name][:, b, :],
                            ident,
                        )
                        if j == 3:
                            dst = t[dst_name]
                            nc.vector.tensor_copy(
                                dst[:, 512 * grp : 512 * (grp + 1)], ptr
                            )

                    return run

                groups = {"vp": [vp_unit]}
                for gname, dst, srcf in (("q", "qt2", "qin"), ("k", "kt2", "kin")):
                    for grp in range(4):
                        groups[f"{gname}{grp}"] = [
                            tr_unit(dst, srcf, grp, j) for j in range(4)
                        ]
                return groups

            def epilogue_units(h, groups):
                """Transpose+normalize groups (2 q-blocks each... 4 blocks)"""
                t = tiles[h]
                osb, ot, rt = t["osb"], t["ot"], t["rt"]
                units = []
                for g in groups:

                    def ep_unit(g=g):
                        trp = trp_pool.tile(
                            [128, 4, DK + 1], f32, tag="tr", name=f"ep{h}{g}"
                        )
                        for j in range(4):
                            b = 4 * g + j
                            nc.tensor.transpose(
                                trp[:, j, :],
                                osb[:, 128 * b : 128 * (b + 1)],
                                ident[0 : DK + 1, 0 : DK + 1],
                            )
                        nc.vector.reciprocal(rt[:, 4 * g : 4 * g + 4], trp[:, :, DK])
                        for j in range(4):
                            b = 4 * g + j
                            nc.vector.tensor_scalar_mul(
                                ot[:, b, :], trp[:, j, 0:DK], rt[:, b : b + 1]
                            )

                    units.append(ep_unit)
                return units

            emit_loads(0)
            g0 = prologue_units(0)
            # upfront: vp + q quarters 0-1 + k quarter 0; the rest
            # interleaves into head 0's own k-loop ordered by first use:
            # k q1 (iter 4), q q2/q3 (pass-1 start, iter 8), k q2/q3
            # (iters 16/20); consumed 2 per iteration.
            for u in g0["vp"] + g0["q0"] + g0["q1"] + g0["k0"]:
                u()
            leftover0 = (
                g0["k1"] + g0["q2"] + g0["q3"] + g0["k2"] + g0["k3"]
            )
            pending_ep = []

            for h in range(HPC):
                t = tiles[h]
                qt2, kt2, vp = t["qt2"], t["kt2"], t["vp"]
                t["osb"] = outp.tile([DK + 1, S], f32, tag="osb", name=f"osb{h}")
                t["ot"] = outp.tile([128, NB, DK], f32, tag="ot", name=f"ot{h}")
                t["rt"] = outp.tile([128, NB], f32, tag="rt", name=f"rt{h}")
                osb = t["osb"]
                pending_pro = []
                it_count = 0

                for wi in range(2):
                    w0 = QW * wi
                    ce = w0 + QW
                    ilist = [
                        i for i in range(NB) if not (causal and w0 + QW <= 128 * i)
                    ]
                    last_i = ilist[-1]
                    oacc = oap.tile([DK + 1, QW], f32, tag="oacc", name=f"oacc{h}{wi}")
                    pending_pv = []

                    def emit_pv(pend, oacc=oacc, w0=w0, first_i=None):
                        pi_, pt_, pieces_, fi = pend
                        for a, bnd, stop_f in pieces_:
                            nc.tensor.matmul(
                                oacc[:, a - w0 : bnd - w0],
                                vp[:, pi_, :],
                                pt_[:, a - w0 : bnd - w0],
                                start=fi,
                                stop=stop_f,
                                skip_group_check=True,
                            )

                    for i in ilist:
                        # interleave cross-head work into the dense stream
                        if it_count == 0:
                            if h == 0:
                                pending_pro = list(leftover0)
                            if h + 1 < HPC:
                                emit_loads(h + 1)
                                gs = prologue_units(h + 1)
                                pending_pro = pending_pro + [
                                    u
                                    for key in (
                                        "vp", "q0", "q1", "q2", "q3",
                                        "k0", "k1", "k2", "k3",
                                    )
                                    for u in gs[key]
                                ]
                        if pending_pro:
                            pending_pro.pop(0)()
                            if (h == 0 or it_count >= 10) and pending_pro:
                                pending_pro.pop(0)()
                        if it_count % 3 == 2 and pending_ep:
                            pending_ep.pop(0)()
                        it_count += 1

                        kstart = 128 * i
                        cs = max(w0, kstart) if causal else w0
                        sc = ps.tile([128, QW], f32, tag="ps", name=f"sc{h}{wi}{i}")
                        pieces = _split_bank_pieces(cs, ce, w0)
                        for pi_, (a, bnd) in enumerate(pieces):
                            has_diag = causal and cs == kstart and pi_ == 0
                            if has_diag:
                                nc.tensor.matmul(
                                    sc[:, a - w0 : bnd - w0],
                                    identb,
                                    maskc[:, 0 : bnd - a],
                                    start=True,
                                    stop=False,
                                    skip_group_check=True,
                                )
                            nc.tensor.matmul(
                                sc[:, a - w0 : bnd - w0],
                                kt2[:, kstart : kstart + 128],
                                qt2[:, a:bnd],
                                start=not has_diag,
                                stop=True,
                                skip_group_check=True,
                            )
                        pt = ptp.tile([128, QW], bf16, tag="pt", name=f"pt{h}{wi}{i}")
                        nc.scalar.activation(
                            pt[:, cs - w0 : QW],
                            sc[:, cs - w0 : QW],
                            mybir.ActivationFunctionType.Exp,
                            scale=scale,
                        )
                        if pending_pv:
                            emit_pv(pending_pv.pop(0))
                        if not sim_safe and i == 8 * wi + 5:
                            # cols [w0, w0+512) got their last PV at k-tile
                            # 8*wi+3 (popped at iteration 8*wi+4): copy that
                            # half out now so epilogue groups start earlier
                            nc.vector.tensor_copy(
                                osb[:, w0 : w0 + 512], oacc[:, 0:512]
                            )
                        if causal and cs == kstart and sim_safe:
                            pv_pieces = [(cs, cs + 128, True)]
                            pv_pieces += [
                                (a, bnd, i == last_i)
                                for a, bnd in _split_bank_pieces(cs + 128, ce, w0)
                            ]
                        else:
                            pv_pieces = [(a, bnd, i == last_i) for a, bnd in pieces]
                        pending_pv.append((i, pt, pv_pieces, i == ilist[0]))
                    for pend in pending_pv:
                        emit_pv(pend)
                    if sim_safe:
                        nc.vector.tensor_copy(osb[:, w0:ce], oacc)
                    else:
                        nc.vector.tensor_copy(
                            osb[:, w0 + 512 : ce], oacc[:, 512:QW]
                        )
                    # epilogue groups for this pass's q-blocks become eligible
                    pending_ep.extend(epilogue_units(h, [2 * wi, 2 * wi + 1]))
                    if wi == 1:
                        hh = h

                        def out_dma(hh=hh):
                            nc.sync.dma_start(
                                o[hh].rearrange("(b p) d -> p b d", p=128),
                                tiles[hh]["ot"],
                            )

                        pending_ep.append(out_dma)

                for u in pending_pro:
                    u()
            for u in pending_ep:
                u()

    nc.compile()
    return nc


def _get_program(causal, scale):
    key = (causal, float(scale))
    if key not in _CACHE:
        _CACHE[key] = _build_program(causal=causal, scale=scale)
    return _CACHE[key]


def _mask_kind(mask):
    """'causal' | 'none' | 'other'"""
    if mask is None:
        return "none"
    m = np.asarray(mask)
    if m.size == 0 or not np.any(m):
        return "none"
    m2 = m.reshape(m.shape[-2], m.shape[-1])
    tri = np.triu(np.ones((S, S), dtype=m2.dtype), k=1)
    if m2.shape == (S, S) and np.array_equal(m2, tri):
        return "causal"
    return "other"


def _host_reference(queries, keys, values, dk, mask):
    """Correctness fallback for mask shapes the device program doesn't cover."""
    q = queries.astype(np.float64)
    kk = keys.astype(np.float64)
    vv = values.astype(np.float64)
    score = np.einsum("bhqd,bhkd->bhqk", q, kk) / np.sqrt(np.float64(dk))
    if mask is not None:
        score = score + np.asarray(mask, dtype=np.float64) * -1e9
    score -= score.max(axis=-1, keepdims=True)
    e = np.exp(score)
    attn = e / e.sum(axis=-1, keepdims=True)
    return np.einsum("bhqk,bhkd->bhqd", attn, vv).astype(np.float32)


def kernel(queries, keys, values, dk, mask=None, **_kw):
    global LAST_RESULT
    dk_val = int(np.asarray(dk))
    kind = _mask_kind(mask)
    if kind == "other":
        return _host_reference(queries, keys, values, dk_val, mask)

    from concourse.bass_utils import run_bass_kernel_spmd

    scale = 1.0 / float(np.sqrt(np.float64(dk_val)))
    nc = _get_program(causal=(kind == "causal"), scale=scale)

    qf = np.ascontiguousarray(
        np.asarray(queries, dtype=np.float32).reshape(B * H, S, DK)
    )
    kf = np.ascontiguousarray(np.asarray(keys, dtype=np.float32).reshape(B * H, S, DK))
    vf = np.ascontiguousarray(
        np.asarray(values, dtype=np.float32).reshape(B * H, S, DK)
    )

    in_maps = [
        {
            "q": qf[HPC * c : HPC * (c + 1)],
            "k": kf[HPC * c : HPC * (c + 1)],
            "v": vf[HPC * c : HPC * (c + 1)],
        }
        for c in range(NCORES)
    ]
    res = run_bass_kernel_spmd(nc, in_maps, core_ids=list(range(NCORES)))
    LAST_RESULT = res
    out = np.stack([res.results[c]["o"] for c in range(NCORES)], axis=0)
    return out.reshape(B, H, S, DK).astype(np.float32)


if __name__ == "__main__":
    # smoke: build the program only
    nc = _build_program()
    print("program built ok")
